# revision 1
# baseline (speedup 1.0000x reference)
"""Trainium2 Bass kernel for nn_EncoderInput (DA-RNN input-attention encoder).

Data-parallel over batch: 8 cores x 16 batch rows each. Full inputs in,
full output out; sharding/marshalling (transposes, bf16 casts, chunk
re-layout) happens host-side, all FLOPs happen on-device.

Per-core algorithm (B=16 shard, T=128 steps, N=128 drivers, M=256 hidden):
  UX[b,n,u]   = sum_t X[b,t,n] Ud[t,u] + bU[u]          (precomputed, PE)
  XW[t,m,b]   = (x_t @ Wx + b)^T                        (precomputed, PE)
  per step t:
    z^T  = XW[t] + Wh^T-chunks @ h^T   (PE, gate-chunk x batch layout)
    i,f,o = 0.5*tanh(z/2)+0.5 ; g = tanh(z)             (ACT + DVE affine)
    c = f*c + i*g ; h = o*tanh(c)                       (DVE, c fp32)
    w^T  = Wd^T-chunks @ [h;c]^T + bW                   (PE + DVE)
    ARG[u,(b,n)] = UX^T + w^T[:,b]                      (16x DVE tensor_scalar)
    TANH = tanh(ARG)                                    (1 big ACT op)
    e    = sum_u vd[u]*TANH  -> PSUM partitions (8b+s)  (16 sparse-vd matmuls)
  per 8-step group: softmax over n (ACT exp + DVE) ; out = X * alpha
"""

import sys

for _p in ("/opt/trn_rl_repo",):
    if _p not in sys.path:
        sys.path.insert(0, _p)

import numpy as np
import ml_dtypes

BF16 = ml_dtypes.bfloat16

import concourse.bass as bass
import concourse.tile as tile
from concourse import bacc, mybir

F32 = mybir.dt.float32
BF = mybir.dt.bfloat16
AF = mybir.ActivationFunctionType
ALU = mybir.AluOpType
AX = mybir.AxisListType

B, T, N, M = 128, 128, 128, 256
NCORES = 8
BS = B // NCORES          # 16 batch rows per core
G = 8                     # steps per softmax group
NG = T // G               # 16 groups
# psum slot -> m-chunk of z (gate order i,f,g,o; we pack i,f,o then g)
PERM = [0, 1, 2, 3, 6, 7, 4, 5]

# software-pipeline lags (in emission windows)
LAG_W = 1      # Wd matmuls of step t emitted in window t+LAG_W
LAG_TANH = 3   # big tanh of step t in window t+LAG_TANH
LAG_E = 5      # e-matmuls of step t in window t+LAG_E
LAG_SM = 13    # softmax tail of group g at window 8g+LAG_SM


def _build_kernel(nc):
    x_d = nc.dram_tensor("x", [BS, T, N], F32, kind="ExternalInput")
    xnb_d = nc.dram_tensor("xnb", [T, BS, N], BF, kind="ExternalInput")
    xtb_d = nc.dram_tensor("xtb", [N, T, BS], BF, kind="ExternalInput")
    wht_d = nc.dram_tensor("wht", [128, 2, 8, 128], BF, kind="ExternalInput")
    wxt_d = nc.dram_tensor("wxt", [128, 8, 128], BF, kind="ExternalInput")
    udt_d = nc.dram_tensor("udt", [T, T], BF, kind="ExternalInput")
    wdt_d = nc.dram_tensor("wdt", [128, 4, T], BF, kind="ExternalInput")
    s_d = nc.dram_tensor("svd", [T, 32, 32], BF, kind="ExternalInput")
    bt_d = nc.dram_tensor("bt", [128, 8], F32, kind="ExternalInput")
    bwc_d = nc.dram_tensor("bwc", [T, 1], F32, kind="ExternalInput")
    buc_d = nc.dram_tensor("buc", [T, 1], F32, kind="ExternalInput")
    h0t_d = nc.dram_tensor("h0t", [128, 32], BF, kind="ExternalInput")
    c0t_d = nc.dram_tensor("c0t", [128, 32], F32, kind="ExternalInput")
    eye_d = nc.dram_tensor("eye", [128, 128], BF, kind="ExternalInput")
    out_d = nc.dram_tensor("out", [BS, T, N], F32, kind="ExternalOutput")

    with tile.TileContext(nc) as tc:
        with tc.tile_pool(name="const", bufs=1) as const:
            # persistent SBUF residents
            uxt = const.tile([128, N, BS], BF)         # [u, n, b]
            xwt = const.tile([128, 8, T, BS], BF)      # [m, slot, t, b]
            wht = const.tile([128, 2, 8, 128], BF)
            wxt = const.tile([128, 8, 128], BF)
            udt = const.tile([T, T], BF)
            wdt = const.tile([128, 4, T], BF)
            svd = const.tile([T, 32, 32], BF)
            bt = const.tile([128, 8], F32)
            bwc = const.tile([T, 1], F32)
            buc = const.tile([T, 1], F32)
            eye = const.tile([128, 128], BF)
            bubw = const.tile([T, 1], F32)
            xnb = const.tile([T, BS, N], BF)
            xtb = const.tile([N, T, BS], BF)
            xga = const.tile([128, 16, N], F32)
            h0 = const.tile([128, 32], BF)
            c0 = const.tile([128, 32], F32)

            for sb, dr in [
                (wht, wht_d), (wxt, wxt_d), (udt, udt_d), (wdt, wdt_d),
                (svd, s_d), (bt, bt_d), (bwc, bwc_d), (buc, buc_d),
                (eye, eye_d), (xnb, xnb_d), (xtb, xtb_d),
                (h0, h0t_d), (c0, c0t_d),
            ]:
                nc.sync.dma_start(out=sb[:], in_=dr.ap())

            for bb in range(BS):
                src_ap = bass.AP(
                    tensor=x_d, offset=bb * T * N,
                    ap=[[N, 8], [8 * N, 16], [1, N]])
                nc.sync.dma_start(out=xga[8 * bb:8 * bb + 8, :, :],
                                  in_=src_ap)

            nc.vector.tensor_add(bubw[:], buc[:], bwc[:])

            # pre-scale g-gate weights x2: gates become tanh(0.5*z)
            nc.vector.tensor_scalar(wht[:, :, 6:8, :], wht[:, :, 6:8, :],
                                    2.0, None, ALU.mult)
            nc.vector.tensor_scalar(wxt[:, 6:8, :], wxt[:, 6:8, :],
                                    2.0, None, ALU.mult)
            nc.vector.tensor_scalar(bt[:, 6:8], bt[:, 6:8], 2.0, None,
                                    ALU.mult)

            # ---------------- precompute ----------------
            with tc.tile_pool(name="pre", bufs=2, space="PSUM") as pre:
                # UX^T: [u,(b,n)] = sum_t Ud[t,u] * X[b,t,n]  (+ bU)
                for q in range(2):
                    ps = pre.tile([128, 1024], F32)
                    for r in range(2):
                        idx = 2 * q + r
                        nc.tensor.matmul(
                            ps[:, 512 * r:512 * (r + 1)],
                            udt[:],
                            xnb[:, 4 * idx:4 * (idx + 1), :],
                            start=True, stop=True,
                        )
                    uxt_dst = bass.AP(
                        tensor=uxt.tensor, offset=uxt.offset + 8 * q,
                        ap=[uxt.ap[0], [1, 8], [BS, N]])
                    nc.scalar.activation(
                        uxt_dst, ps[:].rearrange("p (b n) -> p b n", b=8),
                        AF.Identity, bias=bubw[:, 0:1],
                    )
                # XW^T: [m,(t,b)] = sum_n Wx[n,m] X[b,t,n]  (+ b)
                for sl in range(8):
                    ps = pre.tile([128, 1024], F32)
                    for q in range(2):
                        for r in range(2):
                            tq = 32 * (2 * q + r)
                            nc.tensor.matmul(
                                ps[:, 512 * r:512 * (r + 1)],
                                wxt[:, sl, :],
                                xtb[:, tq:tq + 32, :],
                                start=True, stop=True,
                            )
                        dst = xwt[:, sl, 64 * q:64 * (q + 1), :]
                        if sl % 2 == 0:
                            nc.scalar.activation(
                                dst, ps[:], AF.Identity, bias=bt[:, sl:sl + 1])
                        else:
                            nc.vector.tensor_scalar(
                                dst, ps[:], bt[:, sl:sl + 1], None, ALU.add)

            # ---------------- main loop ----------------
            with (
                tc.tile_pool(name="zps", bufs=3, space="PSUM") as zps,
                tc.tile_pool(name="eps", bufs=2, space="PSUM") as eps,
                tc.tile_pool(name="state", bufs=4) as state,
                tc.tile_pool(name="work", bufs=4) as work,
                tc.tile_pool(name="att", bufs=6) as att,
                tc.tile_pool(name="soft", bufs=2) as soft,
            ):
                h_of = {0: h0}
                cb_of = {}
                c_cur = c0
                w_of = {}     # step -> wT sbuf tile (128, BS)
                arg_of = {}   # step -> ARG tile
                tanh_of = {}  # step -> TANH tile
                e_of = {}     # group -> E psum tile
                xg_of = {}    # group -> X group tile
                wof_pending = {}   # step -> (wT, w_nat) sbuf tiles

                for t in range(T + LAG_SM + (G - 1) * 0 + 4):
                    s, g = t % G, t // G
                    if t <= T - 1:
                        # --- PE: z matmuls of step t ---
                        hp = tc.high_priority(offset=400)
                        hp.__enter__()
                        zt = zps.tile([128, 144], F32)
                        # single full-bank matmul: copies XW for all 8 slots
                        # into PSUM and clears the bank (start=True)
                        nc.tensor.matmul(
                            zt[:, 0:128], eye[:], xwt[:, :, t, :],
                            start=True, stop=False)
                        for sl in range(8):
                            for kc in range(2):
                                nc.tensor.matmul(
                                    zt[:, 16 * sl:16 * (sl + 1)],
                                    wht[:, kc, sl, :],
                                    h_of[t][:, 16 * kc:16 * (kc + 1)],
                                    start=False, stop=(kc == 1))

                        # --- ACT: gates (one instr; g-weights pre-scaled) ---
                        ifog = work.tile([128, 128], BF, tag="ifog")
                        nc.scalar.activation(ifog[:], zt[:, 0:128],
                                             AF.Tanh, scale=0.5)

                        # --- DVE: elementwise LSTM update ---
                        ifo = work.tile([128, 96], BF, tag="ifo")
                        nc.vector.tensor_scalar(ifo[:], ifog[:, 0:96], 0.5, 0.5,
                                                ALU.mult, ALU.add)
                        ig = work.tile([128, 32], F32, tag="ig")
                        nc.vector.tensor_mul(ig[:], ifo[:, 0:32],
                                             ifog[:, 96:128])
                        fc = work.tile([128, 32], F32, tag="fc")
                        nc.vector.tensor_mul(fc[:], ifo[:, 32:64], c_cur[:])
                        c_new = state.tile([128, 32], F32, tag="c")
                        nc.vector.tensor_add(c_new[:], ig[:], fc[:])
                        tc_t = work.tile([128, 32], BF, tag="tc")
                        nc.scalar.activation(tc_t[:], c_new[:], AF.Tanh)
                        h_new = state.tile([128, 32], BF, tag="h")
                        nc.vector.tensor_mul(h_new[:], ifo[:, 64:96], tc_t[:])
                        cb_new = state.tile([128, 32], BF, tag="cb")
                        nc.vector.tensor_copy(cb_new[:], c_new[:])
                        hp.__exit__(None, None, None)
                        h_of[t + 1] = h_new
                        cb_of[t + 1] = cb_new
                        c_cur = c_new

                        # --- PE: w^T matmuls (into ZWW region) ---
                        hs = [h_new[:, 0:16], h_new[:, 16:32],
                              cb_new[:, 0:16], cb_new[:, 16:32]]
                        for kc in range(4):
                            nc.tensor.matmul(
                                zt[:, 128:144], wdt[:, kc, :], hs[kc],
                                start=False, stop=(kc == 3))
                        wt_sb = work.tile([128, BS], BF, tag="wt")
                        nc.vector.tensor_copy(wt_sb[:], zt[:, 128:144])
                        wof_pending[t] = wt_sb

                    # --- broadcast w over n for step t-LAG_W ---
                    tw = t - LAG_W
                    if 0 <= tw <= T - 1:
                        wt_sb = wof_pending.pop(tw)
                        # one 2x-mode TT: stride-0 over n, step-1 over b
                        arg = att.tile([128, N, BS], BF, tag="arg")
                        wt_bc = bass.AP(
                            tensor=wt_sb.tensor, offset=wt_sb.offset,
                            ap=[wt_sb.ap[0], [0, N], [1, BS]])
                        nc.vector.tensor_tensor(
                            arg[:], uxt[:], wt_bc, ALU.add)
                        arg_of[tw] = arg

                    # --- ACT: big tanh of step t-LAG_TANH ---
                    tt = t - LAG_TANH
                    if tt >= 0 and tt in arg_of:
                        th = att.tile([128, N, BS], BF, tag="tanh")
                        argt = arg_of.pop(tt)
                        for c0, c1 in [(0, 64), (64, 128)]:
                            nc.scalar.activation(
                                th[:, c0:c1, :], argt[:, c0:c1, :], AF.Tanh)
                        tanh_of[tt] = th

                    # --- PE: e-matmuls of step t-LAG_E ---
                    te = t - LAG_E
                    if te >= 0 and te in tanh_of:
                        se, ge = te % G, te // G
                        if ge not in e_of:
                            e_of[ge] = eps.tile([128, N], F32, tag="eps",
                                                name="etile")
                            nc.vector.memset(e_of[ge][:], 0.0)
                        ep = e_of[ge]
                        th = tanh_of.pop(te)
                        for bb in range(BS):
                            cg = bb // 4
                            v = (bb % 4) * 8 + se
                            nc.tensor.matmul(
                                ep[32 * cg:32 * (cg + 1), :],
                                svd[:, v, :],
                                th[:, :, bb],
                                start=False,
                                stop=(se == G - 1 and bb % 4 == 3),
                                tile_position=(0, 32 * cg),
                            )

                    # --- softmax + output of group (t-LAG_SM)//G ---
                    if t >= LAG_SM and (t - LAG_SM) % G == 0:
                        gs = (t - LAG_SM) // G
                        if gs in e_of:
                            ep = e_of.pop(gs)
                            ex = soft.tile([128, N], F32, tag="ex")
                            nc.scalar.activation(ex[:], ep[:], AF.Exp)
                            sm = soft.tile([128, 1], F32, tag="sm")
                            nc.vector.tensor_reduce(sm[:], ex[:], axis=AX.X,
                                                    op=ALU.add)
                            rc = soft.tile([128, 1], F32, tag="rc")
                            nc.vector.reciprocal(rc[:], sm[:])
                            al = soft.tile([128, N], F32, tag="al")
                            nc.vector.tensor_scalar(al[:], ex[:], rc[:, 0:1],
                                                    None, ALU.mult)
                            ot = soft.tile([128, N], F32, tag="ot")
                            nc.vector.tensor_mul(ot[:], al[:], xga[:, gs, :])
                            nc.sync.dma_start(
                                out=out_d.ap()[:, G * gs:G * (gs + 1), :],
                                in_=ot[:])
    return nc


_CACHE = {}


def _get_nc():
    if "nc" not in _CACHE:
        nc = bacc.Bacc("TRN2", target_bir_lowering=False, debug=False)
        _build_kernel(nc)
        nc.compile()
        _CACHE["nc"] = nc
    return _CACHE["nc"]


def kernel(X, h0, s0, Wx, Wh, b, Wd, bW, Ud, bU, vd, bv):
    X = np.asarray(X, np.float32)
    h0 = np.asarray(h0, np.float32)
    s0 = np.asarray(s0, np.float32)
    Wx = np.asarray(Wx, np.float32)
    Wh = np.asarray(Wh, np.float32)
    b = np.asarray(b, np.float32)
    Wd = np.asarray(Wd, np.float32)
    bW = np.asarray(bW, np.float32)
    Ud = np.asarray(Ud, np.float32)
    bU = np.asarray(bU, np.float32)
    vd = np.asarray(vd, np.float32)

    # replicated (weight) marshalling — layout only, no FLOPs
    wht = np.ascontiguousarray(
        Wh.reshape(2, 128, 8, 128).transpose(1, 0, 2, 3)[:, :, PERM, :]
    ).astype(BF16)
    wxt = np.ascontiguousarray(Wx.reshape(128, 8, 128)[:, PERM, :]).astype(BF16)
    udt = Ud.astype(BF16)
    wdt = np.ascontiguousarray(Wd.reshape(4, 128, 128).transpose(1, 0, 2)
                               ).astype(BF16)
    svd = np.zeros((128, 32, 32), np.float32)
    for v in range(32):
        svd[:, v, v] = vd[:, 0]
    svd = svd.astype(BF16)
    bt = np.ascontiguousarray(b.reshape(8, 128)[PERM].T).astype(np.float32)
    bwc = bW.reshape(T, 1).astype(np.float32)
    buc = bU.reshape(T, 1).astype(np.float32)
    eye = np.eye(128, dtype=BF16)

    def tr_state(v):  # (16,256) -> (128, 32) with col = 16*j + b
        return np.ascontiguousarray(
            v.T.reshape(2, 128, BS).transpose(1, 0, 2).reshape(128, 2 * BS))

    in_maps = []
    for c in range(NCORES):
        xs = X[BS * c:BS * (c + 1)]
        in_maps.append({
            "x": np.ascontiguousarray(xs),
            "xnb": np.ascontiguousarray(xs.transpose(1, 0, 2)).astype(BF16),
            "xtb": np.ascontiguousarray(xs.transpose(2, 1, 0)).astype(BF16),
            "wht": wht, "wxt": wxt, "udt": udt, "wdt": wdt, "svd": svd,
            "bt": bt, "bwc": bwc, "buc": buc, "eye": eye,
            "h0t": tr_state(h0[BS * c:BS * (c + 1)]).astype(BF16),
            "c0t": tr_state(s0[BS * c:BS * (c + 1)]).astype(np.float32),
        })

    from concourse.bass_utils import run_bass_kernel_spmd
    nc = _get_nc()
    _CACHE["in_maps"] = in_maps
    res = run_bass_kernel_spmd(nc, in_maps, core_ids=list(range(NCORES)))
    out = np.concatenate(
        [np.asarray(res.results[c]["out"]) for c in range(NCORES)], axis=0)
    return out.astype(np.float32)

